# revision 8
# baseline (speedup 1.0000x reference)
"""DeepseekV2Lite grouped-expert MLP (MoE experts) on 8 Trainium2 NeuronCores.

Expert-parallel: core m owns experts [4m, 4m+4). Tokens are pre-sorted by
expert, so each core's tokens are a contiguous slice of x; the "all-to-all"
is host-side slicing. Per expert e:  y = (silu(x @ Wg[e]) * (x @ Wu[e])) @ Wd[e].

Device kernel (per core, SPMD over 8 cores):
  - Activations travel transposed (xT: contraction dim H on SBUF partitions),
    so no on-device transposes are needed.
  - gate/up: stationary = 128x128 weight chunk, moving = 256 tokens of xT;
    accumulate 16 H-chunks in PSUM -> guT[ic] = silu(g)*u, [I-chunk, tokens].
  - down: stationary = guT 128x128 chunk, moving = Wd rows [128, 512];
    accumulate 11 I-chunks -> y in natural [tokens, H] layout.
  - Everything streams as bf16 (PSUM accumulation stays fp32): halves HBM
    traffic vs fp32 (the kernel is HBM-bandwidth-bound) and enables the PE's
    fast-weight-load path so LDWEIGHTS hides behind the matmul stream.
  - Weights/activations are relaid out on the host so every DMA reads long
    per-partition-contiguous runs, keeping the 16 SDMA engines near line rate.
"""

import sys

sys.path.insert(0, "/opt/trn_rl_repo")

import numpy as np
import ml_dtypes

BF16 = np.dtype(ml_dtypes.bfloat16)

E, H, I, T = 32, 2048, 1408, 8192
N_CORES = 8
EPC = E // N_CORES  # experts per core
HC = H // 128  # 16 contraction chunks for gate/up
IC = I // 128  # 11 contraction chunks for down
# I-chunk groups per gate/up weight tile: (first chunk, n chunks) pairs
IGS = [(0, 2), (2, 2), (4, 2), (6, 2), (8, 2), (10, 1)]
FREE_GU = HC * I  # per-expert free extent of the relaid gate/up weight slab
FREE_D = IC * H  # per-expert free extent of the relaid down weight slab
TB = 256  # token block (moving-operand width)

_cache: dict[int, object] = {}


def _build(cap: int):
    import concourse.mybir as mybir
    import concourse.tile as tile
    from concourse import bacc

    f32 = mybir.dt.float32
    bf16 = mybir.dt.bfloat16
    ntb = cap // TB

    nc = bacc.Bacc("TRN2", target_bir_lowering=False, debug=False, num_devices=N_CORES)
    # Host-relaid layouts: partition dim first, free dim contiguous per DMA slab.
    xd = nc.dram_tensor("xd", [128, EPC, HC, cap], bf16, kind="ExternalInput").ap()
    wg = nc.dram_tensor("wg", [EPC, 128, FREE_GU], bf16, kind="ExternalInput").ap()
    wu = nc.dram_tensor("wu", [EPC, 128, FREE_GU], bf16, kind="ExternalInput").ap()
    wd = nc.dram_tensor("wd", [EPC, 128, FREE_D], bf16, kind="ExternalInput").ap()
    y = nc.dram_tensor("y", [EPC * cap, H], bf16, kind="ExternalOutput").ap()

    with tile.TileContext(nc) as tc:
        with (
            tc.tile_pool(name="xp", bufs=2) as xp,
            tc.tile_pool(name="wp", bufs=3) as wp,
            tc.tile_pool(name="wdp", bufs=4) as wdp,
            tc.tile_pool(name="gup", bufs=2) as gup,
            tc.tile_pool(name="tp", bufs=2) as tp,
            tc.tile_pool(name="yp", bufs=2) as yp,
            tc.tile_pool(name="pgp", bufs=2, space="PSUM") as pgp,
            tc.tile_pool(name="pup", bufs=2, space="PSUM") as pup,
            tc.tile_pool(name="pyp", bufs=4, space="PSUM") as pyp,
        ):
            HH = HC // 2  # h-half for finer DMA/compute pipelining
            for e in range(EPC):
                for tb in range(ntb):
                    # x rides the (otherwise idle) gpsimd ring so the first
                    # weight DMAs on the sync ring aren't queued behind it
                    xtb = xp.tile([128, HC, TB], bf16, name="xtb")
                    for half in range(2):
                        nc.gpsimd.dma_start(
                            xtb[:, half * HH : (half + 1) * HH, :],
                            xd[:, e, half * HH : (half + 1) * HH, tb * TB : (tb + 1) * TB],
                        )
                    gu = gup.tile([128, IC, TB], bf16, name="gu")
                    wdqs = []
                    for gi, (ic0, icn) in enumerate(IGS):
                        cw = icn * 128
                        off = ic0 * 128 * HC
                        # gate/up weight tiles in h-halves so the PE pipelines
                        # at sub-HAM-window granularity
                        wgt = wp.tile([128, HC, cw], bf16, name="wgt")
                        wut = wp.tile([128, HC, cw], bf16, name="wut")
                        for wt_, src in ((wgt, wg), (wut, wu)):
                            for half in range(2):
                                nc.sync.dma_start(
                                    wt_[:, half * HH : (half + 1) * HH, :],
                                    src[
                                        e,
                                        :,
                                        off + half * HH * cw : off + (half + 1) * HH * cw,
                                    ].rearrange("p (h c) -> p h c", c=cw),
                                )
                        if gi == 3:
                            # Prefetch all four down-proj slabs on the ACT
                            # ring; they stream alongside the tail of phase A
                            # so the down-proj matmuls never wait on HBM.
                            for hg in range(4):
                                wdq = wdp.tile([128, IC, 512], bf16, name="wdq")
                                nc.scalar.dma_start(
                                    wdq[:],
                                    wd[
                                        e, :, hg * IC * 512 : (hg + 1) * IC * 512
                                    ].rearrange("p (i c) -> p i c", c=512),
                                )
                                wdqs.append(wdq)
                        for li in range(icn):
                            pg = pgp.tile([128, TB], f32, name="pg")
                            pu = pup.tile([128, TB], f32, name="pu")
                            for h in range(HC):
                                nc.tensor.matmul(
                                    pg[:],
                                    wgt[:, h, li * 128 : (li + 1) * 128],
                                    xtb[:, h, :],
                                    start=(h == 0),
                                    stop=(h == HC - 1),
                                )
                            for h in range(HC):
                                nc.tensor.matmul(
                                    pu[:],
                                    wut[:, h, li * 128 : (li + 1) * 128],
                                    xtb[:, h, :],
                                    start=(h == 0),
                                    stop=(h == HC - 1),
                                )
                            ts = tp.tile([128, TB], f32, name="ts")
                            nc.scalar.activation(
                                ts[:], pg[:], mybir.ActivationFunctionType.Silu
                            )
                            nc.vector.tensor_mul(gu[:, ic0 + li, :], ts[:], pu[:])
                    ysb0 = yp.tile([128, H], bf16, name="ysb0")
                    ysb1 = yp.tile([128, H], bf16, name="ysb1")
                    for hg in range(4):
                        wdq = wdqs[hg]
                        for n, ysb in enumerate((ysb0, ysb1)):
                            py = pyp.tile([128, 512], f32, name="py")
                            for ic in range(IC):
                                nc.tensor.matmul(
                                    py[:],
                                    gu[:, ic, n * 128 : (n + 1) * 128],
                                    wdq[:, ic, :],
                                    start=(ic == 0),
                                    stop=(ic == IC - 1),
                                )
                            nc.vector.tensor_copy(
                                ysb[:, hg * 512 : (hg + 1) * 512], py[:]
                            )
                    # y rides the ACT HWDGE ring: queued after this expert's
                    # wd slabs and before the next expert's, so the final
                    # write isn't stuck behind 12MB of weight prefetch
                    for n, ysb in enumerate((ysb0, ysb1)):
                        r0 = e * cap + tb * TB + n * 128
                        nc.scalar.dma_start(y[r0 : r0 + 128, :], ysb[:])
    nc.compile()
    return nc


def _get_nc(cap: int):
    if cap not in _cache:
        _cache[cap] = _build(cap)
    return _cache[cap]


def _relayout_gu(W: np.ndarray) -> np.ndarray:
    """[EPC, H, I] -> [EPC, 128, FREE_GU] with per-(expert, I-group) slabs
    contiguous per partition in (h, c) order."""
    Wt = W.reshape(EPC, HC, 128, I).transpose(0, 2, 1, 3)  # [e, p, h, I]
    blocks = [
        np.ascontiguousarray(Wt[:, :, :, ic0 * 128 : (ic0 + icn) * 128]).reshape(
            EPC, 128, HC * icn * 128
        )
        for ic0, icn in IGS
    ]
    return np.concatenate(blocks, axis=2)


def _relayout_d(W: np.ndarray) -> np.ndarray:
    """[EPC, I, H] -> [EPC, 128, FREE_D] with per-(expert, H-group) slabs
    contiguous per partition in (ic, c) order."""
    Wt = W.reshape(EPC, IC, 128, H).transpose(0, 2, 1, 3)  # [e, p, ic, H]
    blocks = [
        np.ascontiguousarray(Wt[:, :, :, hg * 512 : (hg + 1) * 512]).reshape(
            EPC, 128, IC * 512
        )
        for hg in range(4)
    ]
    return np.concatenate(blocks, axis=2)


def _shard(x, Wg, Wu, Wd, offs):
    """Build per-core input maps; returns (in_maps, bounds, cap)."""
    offs = np.asarray(offs).astype(np.int64)
    starts = np.concatenate([[0], offs[:-1]])
    sizes = (offs - starts).astype(np.int64)
    cap = max(TB, int(-(-int(sizes.max()) // TB) * TB))
    xb = x.astype(BF16)
    in_maps = []
    for m in range(N_CORES):
        xm = np.zeros((128, EPC, HC, cap), BF16)
        for le in range(EPC):
            g = m * EPC + le
            s, sz = int(starts[g]), int(sizes[g])
            if sz:
                # [sz, H] -> [p, h, n]
                xm[:, le, :, :sz] = (
                    np.ascontiguousarray(xb[s : s + sz].T)
                    .reshape(HC, 128, sz)
                    .transpose(1, 0, 2)
                )
        sl = slice(m * EPC, (m + 1) * EPC)
        in_maps.append(
            {
                "xd": xm,
                "wg": _relayout_gu(Wg[sl].astype(BF16)),
                "wu": _relayout_gu(Wu[sl].astype(BF16)),
                "wd": _relayout_d(Wd[sl].astype(BF16)),
            }
        )
    return in_maps, (starts, sizes), cap


def _gather(results, bounds, cap, total):
    starts, sizes = bounds
    out = np.zeros((total, H), np.float32)
    for m in range(N_CORES):
        ym = np.asarray(results[m]["y"]).astype(np.float32)
        for le in range(EPC):
            g = m * EPC + le
            s, sz = int(starts[g]), int(sizes[g])
            if sz:
                out[s : s + sz] = ym[le * cap : le * cap + sz]
    return out


def run(x, Wg, Wu, Wd, grouped_mm_offs, trace=False, tmpdir=None):
    from concourse.bass_utils import run_bass_kernel_spmd

    x = np.asarray(x, np.float32)
    Wg = np.asarray(Wg, np.float32)
    Wu = np.asarray(Wu, np.float32)
    Wd = np.asarray(Wd, np.float32)
    in_maps, bounds, cap = _shard(x, Wg, Wu, Wd, grouped_mm_offs)
    nc = _get_nc(cap)
    res = run_bass_kernel_spmd(
        nc, in_maps, list(range(N_CORES)), trace=trace, tmpdir=tmpdir
    )
    total = int(np.asarray(grouped_mm_offs)[-1])
    out = _gather(res.results, bounds, cap, total)
    return out, res


def kernel(x, Wg, Wu, Wd, grouped_mm_offs):
    out, _ = run(x, Wg, Wu, Wd, grouped_mm_offs)
    return out


# revision 11
# speedup vs baseline: 1.0394x; 1.0394x over previous
"""DeepseekV2Lite grouped-expert MLP (MoE experts) on 8 Trainium2 NeuronCores.

Expert-parallel: core m owns experts [4m, 4m+4). Tokens are pre-sorted by
expert, so each core's tokens are a contiguous slice of x; the "all-to-all"
is host-side slicing. Per expert e:  y = (silu(x @ Wg[e]) * (x @ Wu[e])) @ Wd[e].

Device kernel (per core, SPMD over 8 cores):
  - Activations travel transposed (xT: contraction dim H on SBUF partitions),
    so no on-device transposes are needed.
  - gate/up: stationary = 128x128 weight chunk, moving = 256 tokens of xT;
    accumulate 16 H-chunks in PSUM -> guT[ic] = silu(g)*u, [I-chunk, tokens].
  - down: stationary = guT 128x128 chunk, moving = Wd rows [128, 512];
    accumulate 11 I-chunks -> y in natural [tokens, H] layout.
  - Everything streams as bf16 (PSUM accumulation stays fp32): halves HBM
    traffic vs fp32 (the kernel is HBM-bandwidth-bound) and enables the PE's
    fast-weight-load path so LDWEIGHTS hides behind the matmul stream.
  - Weights/activations are relaid out on the host so every DMA reads long
    per-partition-contiguous runs, keeping the 16 SDMA engines near line rate.
"""

import sys

sys.path.insert(0, "/opt/trn_rl_repo")

import numpy as np
import ml_dtypes

BF16 = np.dtype(ml_dtypes.bfloat16)

E, H, I, T = 32, 2048, 1408, 8192
N_CORES = 8
EPC = E // N_CORES  # experts per core
HC = H // 128  # 16 contraction chunks for gate/up
IC = I // 128  # 11 contraction chunks for down
# I-chunk groups per gate/up weight tile: (first chunk, n chunks) pairs
IGS = [(0, 2), (2, 2), (4, 2), (6, 2), (8, 2), (10, 1)]
FREE_GU = HC * I  # per-expert free extent of the relaid gate/up weight slab
FREE_D = IC * H  # per-expert free extent of the relaid down weight slab
TB = 256  # token block (moving-operand width)

_cache: dict[int, object] = {}


def _build(cap: int):
    import concourse.mybir as mybir
    import concourse.tile as tile
    from concourse import bacc

    f32 = mybir.dt.float32
    bf16 = mybir.dt.bfloat16
    ntb = cap // TB

    nc = bacc.Bacc("TRN2", target_bir_lowering=False, debug=False, num_devices=N_CORES)
    # Host-relaid layouts: partition dim first, free dim contiguous per DMA slab.
    xd = nc.dram_tensor("xd", [128, EPC, HC, cap], bf16, kind="ExternalInput").ap()
    wg = nc.dram_tensor("wg", [EPC, 128, FREE_GU], bf16, kind="ExternalInput").ap()
    wu = nc.dram_tensor("wu", [EPC, 128, FREE_GU], bf16, kind="ExternalInput").ap()
    wd = nc.dram_tensor("wd", [EPC, 128, FREE_D], bf16, kind="ExternalInput").ap()
    y = nc.dram_tensor("y", [EPC * cap, H], bf16, kind="ExternalOutput").ap()

    with tile.TileContext(nc) as tc:
        with (
            tc.tile_pool(name="xp", bufs=2) as xp,
            tc.tile_pool(name="wp", bufs=3) as wp,
            tc.tile_pool(name="wdp", bufs=4) as wdp,
            tc.tile_pool(name="gup", bufs=2) as gup,
            tc.tile_pool(name="tp", bufs=2) as tp,
            tc.tile_pool(name="yp", bufs=2) as yp,
            tc.tile_pool(name="pgp", bufs=2, space="PSUM") as pgp,
            tc.tile_pool(name="pup", bufs=2, space="PSUM") as pup,
            tc.tile_pool(name="pyp", bufs=4, space="PSUM") as pyp,
        ):
            HH = HC // 2  # h-half for finer DMA/compute pipelining
            iters = [(e, tb) for e in range(EPC) for tb in range(ntb)]
            xtbs: dict = {}

            def issue_x(it):
                # x rides the (otherwise idle) gpsimd ring so weight DMAs on
                # the sync ring aren't queued behind it; issued one iteration
                # ahead so it never sits behind the y-write's input wait
                e2, tb2 = it
                xtb = xp.tile([128, HC, TB], bf16, name="xtb")
                for half in range(2):
                    nc.gpsimd.dma_start(
                        xtb[:, half * HH : (half + 1) * HH, :],
                        xd[
                            :, e2, half * HH : (half + 1) * HH,
                            tb2 * TB : (tb2 + 1) * TB,
                        ],
                    )
                xtbs[it] = xtb

            issue_x(iters[0])
            for it_idx, (e, tb) in enumerate(iters):
                if True:
                    xtb = xtbs.pop((e, tb))
                    gu = gup.tile([128, IC, TB], bf16, name="gu")
                    wdqs = []
                    for gi, (ic0, icn) in enumerate(IGS):
                        cw = icn * 128
                        off = ic0 * 128 * HC
                        # gate/up weight tiles in h-halves so the PE pipelines
                        # at sub-HAM-window granularity
                        wgt = wp.tile([128, HC, cw], bf16, name="wgt")
                        wut = wp.tile([128, HC, cw], bf16, name="wut")
                        for wt_, src in ((wgt, wg), (wut, wu)):
                            for half in range(2):
                                nc.sync.dma_start(
                                    wt_[:, half * HH : (half + 1) * HH, :],
                                    src[
                                        e,
                                        :,
                                        off + half * HH * cw : off + (half + 1) * HH * cw,
                                    ].rearrange("p (h c) -> p h c", c=cw),
                                )
                        if gi == 4 and it_idx + 1 < len(iters):
                            issue_x(iters[it_idx + 1])
                        if gi == 3:
                            # Prefetch all four down-proj slabs on the ACT
                            # ring; they stream alongside the tail of phase A
                            # so the down-proj matmuls never wait on HBM.
                            for hg in range(4):
                                wdq = wdp.tile([128, IC, 512], bf16, name="wdq")
                                nc.scalar.dma_start(
                                    wdq[:],
                                    wd[
                                        e, :, hg * IC * 512 : (hg + 1) * IC * 512
                                    ].rearrange("p (i c) -> p i c", c=512),
                                )
                                wdqs.append(wdq)
                        for li in range(icn):
                            pg = pgp.tile([128, TB], f32, name="pg")
                            pu = pup.tile([128, TB], f32, name="pu")
                            for h in range(HC):
                                nc.tensor.matmul(
                                    pg[:],
                                    wgt[:, h, li * 128 : (li + 1) * 128],
                                    xtb[:, h, :],
                                    start=(h == 0),
                                    stop=(h == HC - 1),
                                )
                            for h in range(HC):
                                nc.tensor.matmul(
                                    pu[:],
                                    wut[:, h, li * 128 : (li + 1) * 128],
                                    xtb[:, h, :],
                                    start=(h == 0),
                                    stop=(h == HC - 1),
                                )
                            ts = tp.tile([128, TB], f32, name="ts")
                            nc.scalar.activation(
                                ts[:], pg[:], mybir.ActivationFunctionType.Silu
                            )
                            nc.vector.tensor_mul(gu[:, ic0 + li, :], ts[:], pu[:])
                    ysb0 = yp.tile([128, H], bf16, name="ysb0")
                    ysb1 = yp.tile([128, H], bf16, name="ysb1")
                    for hg in range(4):
                        wdq = wdqs[hg]
                        for n, ysb in enumerate((ysb0, ysb1)):
                            py = pyp.tile([128, 512], f32, name="py")
                            for ic in range(IC):
                                nc.tensor.matmul(
                                    py[:],
                                    gu[:, ic, n * 128 : (n + 1) * 128],
                                    wdq[:, ic, :],
                                    start=(ic == 0),
                                    stop=(ic == IC - 1),
                                )
                            nc.vector.tensor_copy(
                                ysb[:, hg * 512 : (hg + 1) * 512], py[:]
                            )
                    for n, ysb in enumerate((ysb0, ysb1)):
                        r0 = e * cap + tb * TB + n * 128
                        nc.gpsimd.dma_start(y[r0 : r0 + 128, :], ysb[:])
    nc.compile()
    return nc


def _get_nc(cap: int):
    if cap not in _cache:
        _cache[cap] = _build(cap)
    return _cache[cap]


def _relayout_gu(W: np.ndarray) -> np.ndarray:
    """[EPC, H, I] -> [EPC, 128, FREE_GU] with per-(expert, I-group) slabs
    contiguous per partition in (h, c) order."""
    Wt = W.reshape(EPC, HC, 128, I).transpose(0, 2, 1, 3)  # [e, p, h, I]
    blocks = [
        np.ascontiguousarray(Wt[:, :, :, ic0 * 128 : (ic0 + icn) * 128]).reshape(
            EPC, 128, HC * icn * 128
        )
        for ic0, icn in IGS
    ]
    return np.concatenate(blocks, axis=2)


def _relayout_d(W: np.ndarray) -> np.ndarray:
    """[EPC, I, H] -> [EPC, 128, FREE_D] with per-(expert, H-group) slabs
    contiguous per partition in (ic, c) order."""
    Wt = W.reshape(EPC, IC, 128, H).transpose(0, 2, 1, 3)  # [e, p, ic, H]
    blocks = [
        np.ascontiguousarray(Wt[:, :, :, hg * 512 : (hg + 1) * 512]).reshape(
            EPC, 128, IC * 512
        )
        for hg in range(4)
    ]
    return np.concatenate(blocks, axis=2)


def _shard(x, Wg, Wu, Wd, offs):
    """Build per-core input maps; returns (in_maps, bounds, cap)."""
    offs = np.asarray(offs).astype(np.int64)
    starts = np.concatenate([[0], offs[:-1]])
    sizes = (offs - starts).astype(np.int64)
    cap = max(TB, int(-(-int(sizes.max()) // TB) * TB))
    xb = x.astype(BF16)
    in_maps = []
    for m in range(N_CORES):
        xm = np.zeros((128, EPC, HC, cap), BF16)
        for le in range(EPC):
            g = m * EPC + le
            s, sz = int(starts[g]), int(sizes[g])
            if sz:
                # [sz, H] -> [p, h, n]
                xm[:, le, :, :sz] = (
                    np.ascontiguousarray(xb[s : s + sz].T)
                    .reshape(HC, 128, sz)
                    .transpose(1, 0, 2)
                )
        sl = slice(m * EPC, (m + 1) * EPC)
        in_maps.append(
            {
                "xd": xm,
                "wg": _relayout_gu(Wg[sl].astype(BF16)),
                "wu": _relayout_gu(Wu[sl].astype(BF16)),
                "wd": _relayout_d(Wd[sl].astype(BF16)),
            }
        )
    return in_maps, (starts, sizes), cap


def _gather(results, bounds, cap, total):
    starts, sizes = bounds
    out = np.zeros((total, H), np.float32)
    for m in range(N_CORES):
        ym = np.asarray(results[m]["y"]).astype(np.float32)
        for le in range(EPC):
            g = m * EPC + le
            s, sz = int(starts[g]), int(sizes[g])
            if sz:
                out[s : s + sz] = ym[le * cap : le * cap + sz]
    return out


def run(x, Wg, Wu, Wd, grouped_mm_offs, trace=False, tmpdir=None):
    from concourse.bass_utils import run_bass_kernel_spmd

    x = np.asarray(x, np.float32)
    Wg = np.asarray(Wg, np.float32)
    Wu = np.asarray(Wu, np.float32)
    Wd = np.asarray(Wd, np.float32)
    in_maps, bounds, cap = _shard(x, Wg, Wu, Wd, grouped_mm_offs)
    nc = _get_nc(cap)
    res = run_bass_kernel_spmd(
        nc, in_maps, list(range(N_CORES)), trace=trace, tmpdir=tmpdir
    )
    total = int(np.asarray(grouped_mm_offs)[-1])
    out = _gather(res.results, bounds, cap, total)
    return out, res


def kernel(x, Wg, Wu, Wd, grouped_mm_offs):
    out, _ = run(x, Wg, Wu, Wd, grouped_mm_offs)
    return out
